# revision 7
# baseline (speedup 1.0000x reference)
"""Behavior-specific FFN (MoE routing) Trainium2 kernel.

Strategy: expert-parallel with host-side routing. Tokens are gathered by
behavior id on the host (numpy), each behavior's tokens are split across
2 of the 8 NeuronCores, and every core runs a dense 2-layer FFN
(relu(x @ W1 + B1) @ W2 + B2) for its single behavior over its token
shard. The host scatters results back; padding tokens (behavior 0) stay
zero.

Device layout: tokens live on the matmul free (moving) dim, feature dims
on partitions. Layer 1: out[F_tile, tok] += W1[H_tile, F_tile].T @
xT[H_tile, tok]; layer 2 contracts over F the same way. x is fed
pre-transposed ([H, N]) by the host so no on-device transpose is needed.
"""

import numpy as np

_B, _T, _H, _F = 32, 512, 512, 2048
_NB = 4
_P = 128
_NCORES = 8
_TOK_TILE = 512

# Stash of the most recent BassKernelResults (exec_time_ns etc.) for the
# local test harness; harmless in the grading path.
LAST_RESULTS = None

_NC_CACHE = {}


def _token_tiles(n_pad):
    """Chunk n_pad into token tiles: full 512s plus one 128-aligned remainder."""
    tiles = []
    off = 0
    while n_pad - off >= _TOK_TILE:
        tiles.append((off, _TOK_TILE))
        off += _TOK_TILE
    if n_pad - off:
        tiles.append((off, n_pad - off))
    return tiles


def _build(n_pad, mm_dtype_name, repeats=1):
    from contextlib import ExitStack

    import concourse.bass as bass
    import concourse.mybir as mybir
    import concourse.tile as tile
    from concourse import bacc

    f32 = mybir.dt.float32
    mm_dt = getattr(mybir.dt, mm_dtype_name)
    AF = mybir.ActivationFunctionType
    KH = _H // _P   # 4  K-subtiles for layer 1 / M-tiles for layer 2
    MF = _F // _P   # 16 M-tiles for layer 1 / K-subtiles for layer 2

    nc = bacc.Bacc("TRN2", target_bir_lowering=False, debug=False, num_devices=_NCORES)
    xT = nc.dram_tensor("xT", [_H, n_pad], f32, kind="ExternalInput").ap()
    w1 = nc.dram_tensor("w1", [_H, _F], f32, kind="ExternalInput").ap()
    w2 = nc.dram_tensor("w2", [_F, _H], f32, kind="ExternalInput").ap()
    b1 = nc.dram_tensor("b1", [_P, MF], f32, kind="ExternalInput").ap()
    b2 = nc.dram_tensor("b2", [_P, KH], f32, kind="ExternalInput").ap()
    yT = nc.dram_tensor("yT", [_H, n_pad], f32, kind="ExternalOutput").ap()

    def mm(ap):
        return ap if ap.dtype == mm_dt else ap.bitcast(mm_dt)

    with tile.TileContext(nc) as tc, ExitStack() as ctx:
        consts = ctx.enter_context(tc.tile_pool(name="consts", bufs=1))
        xp = ctx.enter_context(tc.tile_pool(name="xp", bufs=3))
        hp = ctx.enter_context(tc.tile_pool(name="hp", bufs=2))
        yp = ctx.enter_context(tc.tile_pool(name="yp", bufs=3))
        pp = ctx.enter_context(tc.tile_pool(name="pp", bufs=4, space="PSUM"))

        w1s = consts.tile([_P, KH, _F], f32)
        w2s = consts.tile([_P, MF, _H], f32)
        nc.sync.dma_start(w1s[:], w1.rearrange("(ko p) f -> p ko f", p=_P))
        nc.sync.dma_start(w2s[:], w2.rearrange("(ko p) h -> p ko h", p=_P))
        b1s = consts.tile([_P, MF], f32)
        nc.sync.dma_start(b1s[:], b1)
        b2s = consts.tile([_P, KH], f32)
        nc.sync.dma_start(b2s[:], b2)

        xTr = xT.rearrange("(ko p) n -> p ko n", p=_P)
        yTr = yT.rearrange("(mo p) n -> p mo n", p=_P)

        assert n_pad % _P == 0
        for _rep in range(repeats):
            for t0, tn in _token_tiles(n_pad):
                sl = slice(t0, t0 + tn)
                xt = xp.tile([_P, KH, tn], f32, tag="xt")
                nc.sync.dma_start(xt[:], xTr[:, :, sl])

                ht = hp.tile([_P, MF, tn], f32, tag="ht")
                for m in range(MF):
                    ps = pp.tile([_P, tn], f32, tag="ps1")
                    for k in range(KH):
                        nc.tensor.matmul(
                            ps[:],
                            mm(w1s[:, k, m * _P:(m + 1) * _P]),
                            mm(xt[:, k, :]),
                            start=(k == 0),
                            stop=(k == KH - 1),
                        )
                    nc.scalar.activation(ht[:, m, :], ps[:], AF.Relu, bias=b1s[:, m:m + 1])

                yt = yp.tile([_P, KH, tn], f32, tag="yt")
                for m2 in range(KH):
                    ps2 = pp.tile([_P, tn], f32, tag="ps2")
                    for k2 in range(MF):
                        nc.tensor.matmul(
                            ps2[:],
                            mm(w2s[:, k2, m2 * _P:(m2 + 1) * _P]),
                            mm(ht[:, k2, :]),
                            start=(k2 == 0),
                            stop=(k2 == MF - 1),
                        )
                    nc.scalar.activation(yt[:, m2, :], ps2[:], AF.Identity, bias=b2s[:, m2:m2 + 1])
                nc.sync.dma_start(yTr[:, :, sl], yt[:])

    nc.compile()
    return nc


_MM_DTYPE = "float32"


def _get_nc(n_pad, mm_dtype_name, repeats=1):
    key = (n_pad, mm_dtype_name, repeats)
    if key not in _NC_CACHE:
        _NC_CACHE[key] = _build(n_pad, mm_dtype_name, repeats)
    return _NC_CACHE[key]


def kernel(x, b_seq, W1, B1, W2, B2, _repeats=1):
    global LAST_RESULTS
    import os

    from concourse.bass_utils import run_bass_kernel_spmd

    x = np.asarray(x)
    flat_x = np.ascontiguousarray(x.reshape(-1, _H), dtype=np.float32)
    bs = np.asarray(b_seq).reshape(-1)

    # Route: behavior b -> cores 2b and 2b+1, tokens split evenly.
    idx_per_core = []
    for b in range(_NB):
        idx = np.nonzero(bs == b + 1)[0]
        h = (len(idx) + 1) // 2
        idx_per_core.append(idx[:h])
        idx_per_core.append(idx[h:])
    nmax = max(len(i) for i in idx_per_core)
    n_pad = max(_P, ((nmax + _P - 1) // _P) * _P)

    mm_dtype = os.environ.get("MM_DTYPE", _MM_DTYPE)
    nc = _get_nc(n_pad, mm_dtype, _repeats)

    in_maps = []
    for c in range(_NCORES):
        beh = c // 2
        idx = idx_per_core[c]
        xT = np.zeros((_H, n_pad), np.float32)
        if len(idx):
            xT[:, :len(idx)] = flat_x[idx].T
        in_maps.append({
            "xT": xT,
            "w1": np.ascontiguousarray(W1[beh], dtype=np.float32),
            "w2": np.ascontiguousarray(W2[beh], dtype=np.float32),
            "b1": np.ascontiguousarray(np.asarray(B1[beh], dtype=np.float32).reshape(_F // _P, _P).T),
            "b2": np.ascontiguousarray(np.asarray(B2[beh], dtype=np.float32).reshape(_H // _P, _P).T),
        })

    res = run_bass_kernel_spmd(nc, in_maps, core_ids=list(range(_NCORES)))
    LAST_RESULTS = res

    out = np.zeros((_B * _T, _H), np.float32)
    for c in range(_NCORES):
        idx = idx_per_core[c]
        if len(idx):
            out[idx] = res.results[c]["yT"][:, :len(idx)].T
    return out.reshape(_B, _T, _H)


# revision 10
# speedup vs baseline: 4.6134x; 4.6134x over previous
"""Behavior-specific FFN (MoE routing) Trainium2 kernel.

Strategy: expert-parallel with host-side routing. Tokens are gathered by
behavior id on the host (numpy), each behavior's tokens are split across
2 of the 8 NeuronCores, and every core runs a dense 2-layer FFN
(relu(x @ W1 + B1) @ W2 + B2) for its single behavior over its token
shard. The host scatters results back; padding tokens (behavior 0) stay
zero.

Device layout: tokens live on the matmul free (moving) dim, feature dims
on partitions. Layer 1: out[F_tile, tok] += W1[H_tile, F_tile].T @
xT[H_tile, tok]; layer 2 contracts over F the same way. x is fed
pre-transposed ([H, N]) by the host so no on-device transpose is needed.
"""

import numpy as np

_B, _T, _H, _F = 32, 512, 512, 2048
_NB = 4
_P = 128
_NCORES = 8
_TOK_TILE = 512

# Stash of the most recent BassKernelResults (exec_time_ns etc.) for the
# local test harness; harmless in the grading path.
LAST_RESULTS = None

_NC_CACHE = {}


def _token_tiles(n_pad):
    """Chunk n_pad into token tiles: full 512s plus one 128-aligned remainder."""
    tiles = []
    off = 0
    while n_pad - off >= _TOK_TILE:
        tiles.append((off, _TOK_TILE))
        off += _TOK_TILE
    if n_pad - off:
        tiles.append((off, n_pad - off))
    return tiles


def _build(n_pad, mm_dtype_name, repeats=1):
    from contextlib import ExitStack

    import concourse.bass as bass
    import concourse.mybir as mybir
    import concourse.tile as tile
    from concourse import bacc

    f32 = mybir.dt.float32
    mm_dt = getattr(mybir.dt, mm_dtype_name)
    AF = mybir.ActivationFunctionType
    KH = _H // _P   # 4  K-subtiles for layer 1 / M-tiles for layer 2
    MF = _F // _P   # 16 M-tiles for layer 1 / K-subtiles for layer 2

    nc = bacc.Bacc("TRN2", target_bir_lowering=False, debug=False, num_devices=_NCORES)
    xT = nc.dram_tensor("xT", [_H, n_pad], mm_dt, kind="ExternalInput").ap()
    w1 = nc.dram_tensor("w1", [_H, _F], mm_dt, kind="ExternalInput").ap()
    w2 = nc.dram_tensor("w2", [_F, _H], mm_dt, kind="ExternalInput").ap()
    b1 = nc.dram_tensor("b1", [_P, MF], f32, kind="ExternalInput").ap()
    b2 = nc.dram_tensor("b2", [_P, KH], f32, kind="ExternalInput").ap()
    yT = nc.dram_tensor("yT", [_H, n_pad], f32, kind="ExternalOutput").ap()

    with tile.TileContext(nc) as tc, ExitStack() as ctx:
        consts = ctx.enter_context(tc.tile_pool(name="consts", bufs=1))
        xp = ctx.enter_context(tc.tile_pool(name="xp", bufs=3))
        hp = ctx.enter_context(tc.tile_pool(name="hp", bufs=2))
        yp = ctx.enter_context(tc.tile_pool(name="yp", bufs=3))
        pp = ctx.enter_context(tc.tile_pool(name="pp", bufs=4, space="PSUM"))

        w1s = consts.tile([_P, KH, _F], mm_dt)
        w2s = consts.tile([_P, MF, _H], mm_dt)
        nc.sync.dma_start(w1s[:], w1.rearrange("(ko p) f -> p ko f", p=_P))
        nc.sync.dma_start(w2s[:], w2.rearrange("(ko p) h -> p ko h", p=_P))
        b1s = consts.tile([_P, MF], f32)
        nc.sync.dma_start(b1s[:], b1)
        b2s = consts.tile([_P, KH], f32)
        nc.sync.dma_start(b2s[:], b2)

        xTr = xT.rearrange("(ko p) n -> p ko n", p=_P)
        yTr = yT.rearrange("(mo p) n -> p mo n", p=_P)

        assert n_pad % _P == 0
        for _rep in range(repeats):
            for t0, tn in _token_tiles(n_pad):
                sl = slice(t0, t0 + tn)
                xt = xp.tile([_P, KH, tn], mm_dt, tag="xt")
                nc.sync.dma_start(xt[:], xTr[:, :, sl])

                ht = hp.tile([_P, MF, tn], mm_dt, tag="ht")
                for m in range(MF):
                    ps = pp.tile([_P, tn], f32, tag="ps1")
                    for k in range(KH):
                        nc.tensor.matmul(
                            ps[:],
                            w1s[:, k, m * _P:(m + 1) * _P],
                            xt[:, k, :],
                            start=(k == 0),
                            stop=(k == KH - 1),
                        )
                    nc.scalar.activation(ht[:, m, :], ps[:], AF.Relu, bias=b1s[:, m:m + 1])

                yt = yp.tile([_P, KH, tn], f32, tag="yt")
                for m2 in range(KH):
                    ps2 = pp.tile([_P, tn], f32, tag="ps2")
                    for k2 in range(MF):
                        nc.tensor.matmul(
                            ps2[:],
                            w2s[:, k2, m2 * _P:(m2 + 1) * _P],
                            ht[:, k2, :],
                            start=(k2 == 0),
                            stop=(k2 == MF - 1),
                        )
                    nc.scalar.activation(yt[:, m2, :], ps2[:], AF.Identity, bias=b2s[:, m2:m2 + 1])
                nc.sync.dma_start(yTr[:, :, sl], yt[:])

    nc.compile()
    return nc


_MM_DTYPE = "float32"


def _get_nc(n_pad, mm_dtype_name, repeats=1):
    key = (n_pad, mm_dtype_name, repeats)
    if key not in _NC_CACHE:
        _NC_CACHE[key] = _build(n_pad, mm_dtype_name, repeats)
    return _NC_CACHE[key]


def _np_mm_dtype(mm_dtype_name):
    if mm_dtype_name == "bfloat16":
        import ml_dtypes

        return ml_dtypes.bfloat16
    return np.float32


def _prepare(x, b_seq, W1, B1, W2, B2, mm_dtype_name):
    """Host-side routing: returns (idx_per_core, n_pad, in_maps)."""
    np_dt = _np_mm_dtype(mm_dtype_name)
    x = np.asarray(x)
    flat_x = np.ascontiguousarray(x.reshape(-1, _H), dtype=np.float32)
    bs = np.asarray(b_seq).reshape(-1)

    # Route: behavior b -> cores 2b and 2b+1, tokens split evenly.
    idx_per_core = []
    for b in range(_NB):
        idx = np.nonzero(bs == b + 1)[0]
        h = (len(idx) + 1) // 2
        idx_per_core.append(idx[:h])
        idx_per_core.append(idx[h:])
    nmax = max(len(i) for i in idx_per_core)
    n_pad = max(_P, ((nmax + _P - 1) // _P) * _P)

    in_maps = []
    for c in range(_NCORES):
        beh = c // 2
        idx = idx_per_core[c]
        xT = np.zeros((_H, n_pad), np_dt)
        if len(idx):
            xT[:, :len(idx)] = flat_x[idx].T.astype(np_dt)
        in_maps.append({
            "xT": xT,
            "w1": np.ascontiguousarray(np.asarray(W1[beh]).astype(np_dt)),
            "w2": np.ascontiguousarray(np.asarray(W2[beh]).astype(np_dt)),
            "b1": np.ascontiguousarray(np.asarray(B1[beh], dtype=np.float32).reshape(_F // _P, _P).T),
            "b2": np.ascontiguousarray(np.asarray(B2[beh], dtype=np.float32).reshape(_H // _P, _P).T),
        })
    return idx_per_core, n_pad, in_maps


def kernel(x, b_seq, W1, B1, W2, B2, _repeats=1):
    global LAST_RESULTS
    import os

    from concourse.bass_utils import run_bass_kernel_spmd

    mm_dtype = os.environ.get("MM_DTYPE", _MM_DTYPE)
    idx_per_core, n_pad, in_maps = _prepare(x, b_seq, W1, B1, W2, B2, mm_dtype)
    nc = _get_nc(n_pad, mm_dtype, _repeats)

    res = run_bass_kernel_spmd(nc, in_maps, core_ids=list(range(_NCORES)))
    LAST_RESULTS = res

    out = np.zeros((_B * _T, _H), np.float32)
    for c in range(_NCORES):
        idx = idx_per_core[c]
        if len(idx):
            out[idx] = res.results[c]["yT"][:, :len(idx)].T
    return out.reshape(_B, _T, _H)
